# revision 7
# baseline (speedup 1.0000x reference)
"""HBV hydrological model RHS kernel for Trainium2 (8 NeuronCores, SPMD).

Computes, for B rows:
  dS [B,5], fluxes [B,12]  =  hbv_rhs(y [B,5], theta [14], climate_t [B,3], dt)

Strategy:
  - Shard the batch dim across 8 cores (row-block per core, 128-row aligned;
    last core overlaps the previous one slightly so every core runs the
    identical SPMD program on an equal-sized slice).
  - Per core, tiles of [128 partitions, TR rows/partition]; y/climate/ds/flux
    stay row-interleaved in SBUF ([128, TR, C]) so every DMA moves
    per-partition-contiguous lines.  Compute reads/writes strided APs
    directly; intermediates live in dense [128, TR] temps.
  - theta / delta_t values are baked into the program as immediates at build
    time (they are replicated scalars).
  - ACT funcs restricted to {Copy, Identity, Relu, Square, Exp, Ln} — all in
    one activation table-set, so no table reloads.  sigmoid -> exp+reciprocal,
    x^(1+alpha) -> exp/ln.
"""

import os
import sys

sys.path.insert(0, "/opt/trn_rl_repo")
os.environ.setdefault("MYCRO_LOCAL_CACHE", "1")

import numpy as np

import concourse.bass as bass
import concourse.bacc as bacc
import concourse.tile as tile
from concourse import mybir
from concourse.bass_utils import run_bass_kernel_spmd

F32 = mybir.dt.float32
P = 128
N_CORES = 8
ALU = mybir.AluOpType
AF = mybir.ActivationFunctionType


def _plan(B):
    """Split B rows into 8 equal 128-aligned slices (last may overlap)."""
    chunks = -(-B // P)  # ceil
    per_core = -(-chunks // N_CORES) * P  # rows per core, 128-aligned
    starts = [min(i * per_core, B - per_core) for i in range(N_CORES)]
    assert starts[0] == 0 and all(s >= 0 for s in starts)
    return per_core, starts


def _tile_sizes(rows_c, max_tr=512):
    """Split rows_c (multiple of 128) into per-partition tile lengths."""
    chunks = rows_c // P
    n_tiles = -(-chunks // max_tr)
    base = chunks // n_tiles
    rem = chunks - base * n_tiles
    return [base + (1 if i < rem else 0) for i in range(n_tiles)]


def _build(rows_c, theta, dt, max_tr=512):
    (tt, tti, ttm, cfr, cfmax, whc, cflux, fc, lp, beta, k0, alpha, perc_c,
     k1) = [float(v) for v in theta]
    dt = float(dt)
    idt = 1.0 / dt
    r_, e_ = 0.01, 5.0
    c95 = 1.0 - r_ * e_  # logistic threshold: S2 - Smax + r*e*Smax

    nc = bacc.Bacc("TRN2", target_bir_lowering=False, debug=False,
                   num_devices=N_CORES)
    y_d = nc.dram_tensor("y", [rows_c, 5], F32, kind="ExternalInput").ap()
    c_d = nc.dram_tensor("c", [rows_c, 3], F32, kind="ExternalInput").ap()
    ds_d = nc.dram_tensor("ds", [rows_c, 5], F32, kind="ExternalOutput").ap()
    fx_d = nc.dram_tensor("fx", [rows_c, 12], F32, kind="ExternalOutput").ap()

    trs = _tile_sizes(rows_c, max_tr)

    act = nc.scalar.activation
    vtt = nc.vector.tensor_tensor
    gtt = nc.gpsimd.tensor_tensor

    with tile.TileContext(nc) as tc:
        from contextlib import ExitStack
        with ExitStack() as ctx:
            io = ctx.enter_context(tc.tile_pool(name="io", bufs=2))
            tp = ctx.enter_context(tc.tile_pool(name="tmp", bufs=2))

            base = 0
            for tr in trs:
                rows = P * tr
                sl = slice(base, base + rows)
                Y = io.tile([P, tr, 5], F32, tag="Y")
                C = io.tile([P, tr, 3], F32, tag="C")
                DS = io.tile([P, tr, 5], F32, tag="DS")
                FX = io.tile([P, tr, 12], F32, tag="FX")
                nc.sync.dma_start(
                    out=Y[:], in_=y_d[sl, :].rearrange("(p r) c -> p r c", p=P))
                nc.sync.dma_start(
                    out=C[:], in_=c_d[sl, :].rearrange("(p r) c -> p r c", p=P))

                S1, S2, S3, S4, S5 = (Y[:, :, i] for i in range(5))
                Pr, Ep, T = (C[:, :, i] for i in range(3))
                (sf, refr, melt, rf, inf, se, cf, ea, rr, q0, perc,
                 q1) = (FX[:, :, i] for i in range(12))
                DS1, DS2, DS3, DS4, DS5 = (DS[:, :, i] for i in range(5))

                def tmp(tag):
                    return tp.tile([P, tr], F32, tag=tag, name=tag)

                # dt-scaled storages (dense copies only when dt != 1)
                if dt == 1.0:
                    s1dt, s2dt, s3dt, s4dt = S1, S2, S3, S4
                else:
                    s1dt, s2dt, s3dt, s4dt = (tmp(f"sdt{i}") for i in range(4))
                    for dst, src in ((s1dt, S1), (s2dt, S2), (s3dt, S3),
                                     (s4dt, S4)):
                        act(dst, src, AF.Copy, scale=idt)

                vts = nc.vector.tensor_scalar
                gts = nc.gpsimd.tensor_scalar
                stt = nc.vector.scalar_tensor_tensor

                # ---- snowfall: sf = min(P, relu(P*u)) == P*clamp(u,0,1)
                #      (P >= 0), u = (tt+.5tti-T)/tti
                tA = tmp("A")
                act(tA, T, AF.Copy, scale=-1.0 / tti, bias=(tt + 0.5 * tti) / tti)
                vts(tA, tA, 0.0, 1.0, ALU.max, ALU.min)
                vtt(sf, tA, Pr, ALU.mult)

                # ---- rainfall: rf = P*clamp((T-(tt-.5tti))/tti, 0, 1)
                tD = tmp("D")
                act(tD, T, AF.Copy, scale=1.0 / tti, bias=-(tt - 0.5 * tti) / tti)
                vts(tD, tD, 0.0, 1.0, ALU.max, ALU.min)
                vtt(rf, tD, Pr, ALU.mult)

                # ---- refreeze: refr = relu(min(cfr*cfmax*(ttm-T), S2/dt))
                #      == min(relu(x), S2/dt) since S2 >= 0
                tB = tmp("B")
                act(tB, T, AF.Copy, scale=-cfr * cfmax, bias=cfr * cfmax * ttm)
                stt(refr, tB, 0.0, s2dt, ALU.max, ALU.min)

                # ---- melt: melt = min(relu(cfmax*(T-ttm)), S1/dt)
                tC = tmp("Cm")
                act(tC, T, AF.Copy, scale=cfmax, bias=-cfmax * ttm)
                stt(melt, tC, 0.0, s1dt, ALU.max, ALU.min)

                # ---- infiltration:
                # smax = relu(whc*S1); den = max(r, r*smax); z=(S2-.95smax)/den
                # in = (rf+melt) * sigmoid(z)   [sigmoid via exp+recip]
                tE = tmp("E")  # smax
                act(tE, S1, AF.Relu, scale=whc)
                tF = tmp("F")  # den -> 1/den
                gts(tF, tE, r_, r_, ALU.mult, ALU.max)
                nc.vector.reciprocal(tF, tF)
                tG = tmp("G")  # num -> z -> exp(-z) -> sig
                stt(tG, tE, -c95, S2, ALU.mult, ALU.add)
                vtt(tG, tG, tF, ALU.mult)
                act(tG, tG, AF.Exp, scale=-1.0)
                gts(tG, tG, 1.0, None, ALU.add)
                nc.vector.reciprocal(tG, tG)
                tH = tmp("H")  # rm = rf + melt
                gtt(tH, rf, melt, ALU.add)
                gtt(inf, tH, tG, ALU.mult)

                # ---- excess: se = relu((S2 - whc*S1)/dt)
                tI = tmp("I")
                stt(tI, S1, -whc, S2, ALU.mult, ALU.add)
                act(se, tI, AF.Relu, scale=idt)

                # ---- capillary: cf = min(cflux*(1 - S3/fc), S4/dt)
                tK = tmp("K")
                act(tK, S3, AF.Copy, scale=-cflux / fc, bias=cflux)
                vtt(cf, tK, s4dt, ALU.min)

                # ---- evap: ea = min(Ep*min(S3/(lp*fc), 1), S3/dt)  (Ep >= 0)
                tL = tmp("L")
                vts(tL, S3, 1.0 / (lp * fc), 1.0, ALU.mult, ALU.min)
                vtt(tL, tL, Ep, ALU.mult)
                vtt(ea, tL, s3dt, ALU.min)

                # ---- recharge: r = (in+se) * (relu(S3)/fc)^beta
                tM = tmp("M")
                act(tM, S3, AF.Relu, scale=1.0 / fc)
                if beta == 2.0:
                    act(tM, tM, AF.Square)
                elif beta == 1.0:
                    pass
                else:
                    vts(tM, tM, 1e-30, None, ALU.max)
                    act(tM, tM, AF.Ln)
                    act(tM, tM, AF.Exp, scale=beta)
                tN = tmp("N")  # ise = in + se
                gtt(tN, inf, se, ALU.add)
                gtt(rr, tN, tM, ALU.mult)

                # ---- interflow: q0 = min(k0*relu(S4)^(1+alpha), relu(S4/dt))
                tO = tmp("O")  # relu(S4)
                act(tO, S4, AF.Relu)
                tQ = tmp("Q")
                vts(tQ, tO, 1e-30, None, ALU.max)
                if k0 > 0:
                    # k0 * x^(1+a) == (k0^(1/(1+a)) * x)^(1+a), folded into Ln
                    act(tQ, tQ, AF.Ln, scale=float(k0 ** (1.0 / (1.0 + alpha))))
                    act(tQ, tQ, AF.Exp, scale=1.0 + alpha)
                else:
                    act(tQ, tQ, AF.Ln)
                    act(tQ, tQ, AF.Exp, scale=1.0 + alpha)
                    vts(tQ, tQ, k0, None, ALU.mult)
                if dt == 1.0:
                    vtt(q0, tQ, tO, ALU.min)
                else:
                    tR = tmp("R")
                    act(tR, S4, AF.Relu, scale=idt)
                    vtt(q0, tQ, tR, ALU.min)

                # ---- percolation: perc = min(perc_c, S4/dt)
                if dt == 1.0:
                    gts(perc, S4, perc_c, None, ALU.min)
                else:
                    gts(perc, S4, idt, perc_c, ALU.mult, ALU.min)

                # ---- baseflow: q1 = k1 * S5
                act(q1, S5, AF.Copy, scale=k1)

                # ---- dS
                vtt(DS1, sf, refr, ALU.add)
                vtt(DS1, DS1, melt, ALU.subtract)
                gtt(DS2, tH, refr, ALU.subtract)
                gtt(DS2, DS2, tN, ALU.subtract)
                vtt(DS3, tN, cf, ALU.add)
                vtt(DS3, DS3, ea, ALU.subtract)
                vtt(DS3, DS3, rr, ALU.subtract)
                gtt(DS4, rr, cf, ALU.subtract)
                gtt(DS4, DS4, q0, ALU.subtract)
                gtt(DS4, DS4, perc, ALU.subtract)
                gtt(DS5, perc, q1, ALU.subtract)

                nc.sync.dma_start(
                    out=ds_d[sl, :].rearrange("(p r) c -> p r c", p=P),
                    in_=DS[:])
                nc.sync.dma_start(
                    out=fx_d[sl, :].rearrange("(p r) c -> p r c", p=P),
                    in_=FX[:])
                base += rows
    nc.compile()
    return nc


def kernel(y, theta, climate_data, delta_t, t, _trace=False, _tmpdir=None,
           _max_tr=512):
    y = np.ascontiguousarray(np.asarray(y, dtype=np.float32))
    theta = np.asarray(theta, dtype=np.float32)
    ct = np.ascontiguousarray(np.asarray(climate_data, np.float32)[int(t)])
    dt = float(np.asarray(delta_t, np.float32))

    B = y.shape[0]
    rows_c, starts = _plan(B)
    nc = _build(rows_c, theta, dt, max_tr=_max_tr)

    in_maps = [{"y": y[s:s + rows_c], "c": ct[s:s + rows_c]} for s in starts]
    res = run_bass_kernel_spmd(nc, in_maps, list(range(N_CORES)),
                               trace=_trace, tmpdir=_tmpdir)

    ds = np.empty((B, 5), np.float32)
    fx = np.empty((B, 12), np.float32)
    for i, s in enumerate(starts):
        lo = 0 if i == 0 else max(starts[i - 1] + rows_c - s, 0)
        ds[s + lo:s + rows_c] = res.results[i]["ds"][lo:]
        fx[s + lo:s + rows_c] = res.results[i]["fx"][lo:]
    kernel.last_results = res
    return ds, fx


kernel.last_results = None
